# revision 3
# baseline (speedup 1.0000x reference)
"""FANMoE HyperNet layer on 8 TRN2 NeuronCores — v2 (single fp16 term,
h folded into the matmul stationary).

Algebra: dyn[b,o] = sum_k h[b,k] * (x @ W2_k)[b,o]
                  = sum_k sum_i (x[b,i]*h[b,k]) * W2_k[i,o]
Per hypernet unit k we build xh_k[i,b] = xT[i,b]*h[b,k] in fp16 on the
vector engine (h replicated across partitions by a rank-1 PE outer
product) and use it as the matmul *stationary* against the fp16 W2_k
slice. PSUM then accumulates across all 64 k plus the fp32 base/bias
terms — no per-k vector-engine combine at all. Numerics: single fp16
term measures rel_fro ~4e-4 in simulation (gate is 2e-2).

Sharding: 4 expert-pairs x 2 batch-halves. Each core: 2 experts
(384 output cols), 256 samples, streams its 12.6MB fp16 weight slice
once from HBM.
"""
import math

import numpy as np

import concourse.bass as bass
import concourse.tile as tile
from concourse import mybir, bacc
from concourse.masks import make_identity

B, IN, OUT, COND, N, H = 512, 256, 256, 128, 8, 64
DP = 64
DN = 128
TPE = IN * DP + IN * DN + DN
BH = B // 2          # samples per core (batch half)
NBT = BH // 128      # 128-row tiles per core (=2)
W = 2 * (DP + DN)    # per-core output width: 2 experts x 192 = 384
KSLAB = 8            # hypernet units per weight DMA slab
NSLAB = H // KSLAB
dt = mybir.dt
F32 = dt.float32
F16 = dt.float16
AF = mybir.ActivationFunctionType
OP = mybir.AluOpType
INV2PI = 1.0 / (2.0 * math.pi)
N2PI = -2.0 * math.pi

_cache = {}


def _build(terms=None, repeat_main=1, ablate=()):
    nc = bacc.Bacc("TRN2", target_bir_lowering=False, debug=False)

    def din(name, shape, dty=F32):
        return nc.dram_tensor(name, shape, dty, kind="ExternalInput").ap()

    xh16 = din("xh16", (2, 128, BH), F16)
    condT = din("condT", (COND, BH))
    w2f16 = din("w2f16", (NSLAB, 128, KSLAB, 2, W), F16)
    w2b16 = din("w2b16", (2, H + 1, DN), F16)
    wb16 = din("wb16", (2, 128, W), F16)
    hW1 = din("hW1", (COND, H))
    hb1c = din("hb1c", (H, 1))
    gW1 = din("gW1", (COND, 3 * N))
    gb1 = din("gb1", (3 * N, 1))
    gW2 = din("gW2", (3 * N, N))
    gb2 = din("gb2", (1, N))
    out = nc.dram_tensor("out", (BH, OUT), F32, kind="ExternalOutput").ap()

    with tile.TileContext(nc) as tc:
        with tc.tile_pool(name="const", bufs=1) as cp, \
             tc.tile_pool(name="tmp", bufs=4) as tp:
            ones32 = cp.tile([1, 128], F32)
            nc.vector.memset(ones32, 1.0)
            ones16 = cp.tile([1, 128], F16)
            nc.vector.memset(ones16, 1.0)
            halfpi = cp.tile([128, 1], F32)
            nc.vector.memset(halfpi, math.pi / 2)

            sxh = cp.tile([128, 2, BH], F16)
            for c in range(2):
                nc.sync.dma_start(sxh[:, c, :], xh16[c])
            scT = cp.tile([COND, BH], F32)
            nc.sync.dma_start(scT, condT)
            swb = cp.tile([128, 2, W], F16)
            for c in range(2):
                nc.sync.dma_start(swb[:, c, :], wb16[c])
            sw2b = cp.tile([H + 1, 2, DN], F16)
            for e in range(2):
                nc.sync.dma_start(sw2b[:, e, :], w2b16[e])
            shW1 = cp.tile([COND, H], F32)
            nc.sync.dma_start(shW1, hW1)
            shb1c = cp.tile([H, 1], F32)
            nc.sync.dma_start(shb1c, hb1c)
            sgW1 = cp.tile([COND, 3 * N], F32)
            nc.sync.dma_start(sgW1, gW1)
            sgb1 = cp.tile([3 * N, 1], F32)
            nc.sync.dma_start(sgb1, gb1)
            sgW2 = cp.tile([3 * N, N], F32)
            nc.sync.dma_start(sgW2, gW2)
            sgb2 = cp.tile([1, N], F32)
            nc.sync.dma_start(sgb2, gb2)

            hT16 = cp.tile([H, BH], F16)
            hTflat = cp.tile([1, H, BH], F16)
            hTa = cp.tile([H + 1, BH], F16)
            nc.vector.memset(hTa[H:H + 1, :], 1.0)
            gw_sb = [cp.tile([128, 2], F32, name=f"gw{t}") for t in range(NBT)]
            outf = [cp.tile([128, OUT], F32, name=f"of{t}") for t in range(NBT)]

            # ---------------- prologue: hypernet hT, gating -----------------
            with tc.tile_pool(name="pps", bufs=2, space="PSUM") as pps:
                hp = pps.tile([H, BH], F32, tag="hp", bufs=1)
                nc.tensor.matmul(hp, shW1, scT, start=True, stop=True)
                nc.scalar.activation(hT16, hp, AF.Relu, bias=shb1c)
                nc.scalar.activation(hTa[0:H, :], hp, AF.Relu, bias=shb1c)
                # single-partition copy of hT16 so any k-row can be the
                # moving operand of the rank-1 broadcast matmuls
                nc.sync.dma_start(hTflat, hT16)

                g1 = pps.tile([3 * N, BH], F32, tag="g1", bufs=1)
                nc.tensor.matmul(g1, sgW1, scT, start=True, stop=True)
                g1s = cp.tile([3 * N, BH], F32)
                nc.scalar.activation(g1s, g1, AF.Relu, bias=sgb1)
                for bt in range(NBT):
                    bs = slice(bt * 128, bt * 128 + 128)
                    lg = pps.tile([128, N], F32, tag="lg", bufs=1)
                    nc.tensor.matmul(lg, g1s[:, bs], sgW2,
                                     start=True, stop=False)
                    nc.tensor.matmul(lg, ones32, sgb2, start=False, stop=True)
                    nmx = tp.tile([128, 1], F32, tag="nmx")
                    nc.vector.tensor_reduce(nmx, lg, axis=mybir.AxisListType.X,
                                            op=OP.max, negate=True)
                    ex = tp.tile([128, N], F32, tag="ex")
                    nc.scalar.activation(ex, lg, AF.Exp, bias=nmx)
                    sm = tp.tile([128, 1], F32, tag="sm")
                    nc.vector.tensor_reduce(sm, ex, axis=mybir.AxisListType.X,
                                            op=OP.add)
                    rv = tp.tile([128, 1], F32, tag="rv")
                    nc.vector.reciprocal(rv, sm)
                    nc.vector.tensor_scalar_mul(gw_sb[bt], ex[:, 0:2], rv)

            # ---------------- main: base + sum_k xh_k @ W2_k ---------------
            with tc.tile_pool(name="wp", bufs=3) as wp, \
                 tc.tile_pool(name="xp", bufs=6) as xp, \
                 tc.tile_pool(name="hs", bufs=3) as hsp, \
                 tc.tile_pool(name="hrps", bufs=3, space="PSUM") as hrps, \
                 tc.tile_pool(name="mps", bufs=1, space="PSUM") as mps:

              AHEAD = 4  # k-lookahead of the hrep/xh producer over the MMs

              def _main_body():
                mm = nc.tensor.matmul
                ps = [mps.tile([128, W], F32, tag=f"ps{bt}", bufs=2,
                               name=f"ps{bt}") for bt in range(NBT)]
                wts = []
                for s in range(NSLAB):
                    wt = wp.tile([128, KSLAB, 2, W], F16, tag="w",
                                 name=f"w{s}")
                    if "dma" not in ablate:
                        nc.sync.dma_start(wt, w2f16[s])
                    else:
                        nc.gpsimd.memset(wt, 0.0)
                    wts.append(wt)

                # fp16 base + bias terms open the accumulation groups
                for bt in range(NBT):
                    bs = slice(bt * 128, bt * 128 + 128)
                    mm(ps[bt], sxh[:, 0, bs], swb[:, 0, :],
                       start=True, stop=False)
                    mm(ps[bt], sxh[:, 1, bs], swb[:, 1, :],
                       start=False, stop=False)
                    mm(ps[bt][:, 2 * DP:W], hTa[:, bs], sw2b[:, :, :],
                       start=False, stop=False)

                hs_t, xh_t = {}, {}
                for t in range(H + AHEAD):
                    kp = t
                    if kp < H:
                        if kp % 2 == 0:
                            hpair = hrps.tile([128, 2, BH], F32, tag="hr")
                            hs = hsp.tile([128, 2, BH], F16, tag="hs")
                            hs_t[kp // 2] = hs
                            if "stt" not in ablate:
                                mm(hpair, ones16, hTflat[:, kp:kp + 2, :],
                                   start=True, stop=True)
                                nc.scalar.copy(hs, hpair)
                        xh = xp.tile([128, 2, BH], F16, tag="xh")
                        xh_t[kp] = xh
                        if "stt" not in ablate:
                            hsl = hs_t[kp // 2][:, kp % 2, :]
                            hsb = hsl.unsqueeze(1).broadcast_to((128, 2, BH))
                            nc.vector.scalar_tensor_tensor(
                                xh, sxh, 1.0, hsb, op0=OP.mult, op1=OP.mult)
                        else:
                            nc.vector.memset(xh, 0.0)
                    kc = t - AHEAD
                    if kc < 0 or "mm" in ablate:
                        continue
                    s, kk = kc // KSLAB, kc % KSLAB
                    last = kc == H - 1
                    for bt in range(NBT):
                        bs = slice(bt * 128, bt * 128 + 128)
                        mm(ps[bt], xh_t[kc][:, 0, bs], wts[s][:, kk, 0, :],
                           start=False, stop=False)
                        mm(ps[bt], xh_t[kc][:, 1, bs], wts[s][:, kk, 1, :],
                           start=False, stop=last)

                # ---------- epilogue: sin/cos/relu, gate, store ------------
                def sin_reduced(v, outname, phase):
                    # t1 = (v + phase)/2pi on ACT; round-trip via int32;
                    # residual + range clamps on DVE; Sin(+phase) on ACT.
                    t1 = tp.tile([128, DP], F32, tag="t1")
                    nc.scalar.activation(t1, v, AF.Copy, scale=INV2PI,
                                         bias=phase * INV2PI)
                    ti = tp.tile([128, DP], dt.int32, tag="ti")
                    nc.scalar.copy(ti, t1)
                    tf = tp.tile([128, DP], F32, tag="tf")
                    nc.scalar.copy(tf, ti)
                    r = tp.tile([128, DP], F32, tag="r")
                    nc.vector.scalar_tensor_tensor(r, tf, N2PI, v,
                                                   op0=OP.mult, op1=OP.add)
                    # ACT float->int rounds to nearest, so r + phase is
                    # already in [-pi, pi]: no clamp passes needed
                    sv = tp.tile([128, DP], F32, tag=outname, name=outname)
                    nc.scalar.activation(sv, r, AF.Sin,
                                         bias=halfpi if phase else 0.0)
                    return sv

                for bt in range(NBT):
                    for e in range(2):
                        th = ps[bt][:, e * DP:(e + 1) * DP]
                        g = gw_sb[bt][:, e:e + 1]

                        sv = sin_reduced(th, "sv", 0.0)
                        cv = sin_reduced(th, "cv", math.pi / 2)

                        nn = tp.tile([128, DN], F32, tag="nn")
                        nc.scalar.activation(
                            nn, ps[bt][:, 2 * DP + e * DN:2 * DP + (e + 1) * DN],
                            AF.Relu)

                        if e == 0:
                            nc.vector.tensor_scalar_mul(outf[bt][:, 0:DP],
                                                        cv, g)
                            nc.vector.tensor_scalar_mul(
                                outf[bt][:, DP:2 * DP], sv, g)
                            nc.vector.tensor_scalar_mul(
                                outf[bt][:, 2 * DP:OUT], nn, g)
                        else:
                            stt = nc.vector.scalar_tensor_tensor
                            stt(outf[bt][:, 0:DP], cv, g, outf[bt][:, 0:DP],
                                op0=OP.mult, op1=OP.add)
                            stt(outf[bt][:, DP:2 * DP], sv, g,
                                outf[bt][:, DP:2 * DP],
                                op0=OP.mult, op1=OP.add)
                            stt(outf[bt][:, 2 * DP:OUT], nn, g,
                                outf[bt][:, 2 * DP:OUT],
                                op0=OP.mult, op1=OP.add)
                    nc.sync.dma_start(out[bt * 128:bt * 128 + 128, :],
                                      outf[bt])

              if repeat_main == 1:
                  _main_body()
              else:
                  with tc.For_i(0, repeat_main, 1):
                      _main_body()

    nc.finalize()
    return nc


def _host_prep(x, cond, base_wp, base_wn, base_bn, hW1, hb1, hW2, hb2,
               gW1, gb1, gW2, gb2, terms=None):
    """Build the 8 per-core input maps (layout prep + sharding only)."""
    f32 = np.float32
    f16 = np.float16
    W2r = np.asarray(hW2, f32).reshape(H, N, TPE)
    wpW = W2r[:, :, :IN * DP].reshape(H, N, IN, DP)
    wnW = W2r[:, :, IN * DP:IN * DP + IN * DN].reshape(H, N, IN, DN)
    bnW = W2r[:, :, IN * DP + IN * DN:]                    # (H, N, DN)
    hb2r = np.asarray(hb2, f32).reshape(N, TPE)
    hwp = hb2r[:, :IN * DP].reshape(N, IN, DP)
    hwn = hb2r[:, IN * DP:IN * DP + IN * DN].reshape(N, IN, DN)
    hbn = hb2r[:, IN * DP + IN * DN:]                      # (N, DN)

    base_wp = np.asarray(base_wp, f32)
    base_wn = np.asarray(base_wn, f32)
    base_bn = np.asarray(base_bn, f32)
    x = np.asarray(x, f32)
    cond = np.asarray(cond, f32)
    gW2 = np.asarray(gW2, f32)
    gb2 = np.asarray(gb2, f32)

    common = dict(
        hW1=np.ascontiguousarray(hW1, f32),
        hb1c=np.asarray(hb1, f32).reshape(H, 1).copy(),
        gW1=np.ascontiguousarray(gW1, f32),
        gb1=np.asarray(gb1, f32).reshape(3 * N, 1).copy(),
    )

    halves = []
    for hb in range(2):
        bs = slice(hb * BH, (hb + 1) * BH)
        xT = np.ascontiguousarray(x[bs].T)                 # (IN, BH)
        halves.append(dict(
            xh16=np.ascontiguousarray(xT.astype(f16).reshape(2, 128, BH)),
            condT=np.ascontiguousarray(cond[bs].T),
        ))

    pairs = []
    for p in range(4):
        e0, e1 = 2 * p, 2 * p + 1
        # columns: [wp e0 | wp e1 | wn e0 | wn e1] = 64+64+128+128
        cat = np.concatenate(
            [wpW[:, e0], wpW[:, e1], wnW[:, e0], wnW[:, e1]],
            axis=-1)                                       # (H, IN, 384)
        w2 = cat.reshape(NSLAB, KSLAB, 2, 128, W)
        w2 = np.ascontiguousarray(
            w2.transpose(0, 3, 1, 2, 4).astype(f16))       # (8,128,8,2,384)
        w2bd = np.stack([
            np.concatenate([bnW[:, e], (base_bn[e] + hbn[e])[None, :]], axis=0)
            for e in (e0, e1)])                            # (2, 65, DN)
        wb = np.concatenate(
            [base_wp[e0] + hwp[e0], base_wp[e1] + hwp[e1],
             base_wn[e0] + hwn[e0], base_wn[e1] + hwn[e1]],
            axis=-1)                                       # (IN, 384)
        perm = [e0, e1] + [j for j in range(N) if j not in (e0, e1)]
        pairs.append(dict(
            w2f16=w2,
            w2b16=np.ascontiguousarray(w2bd.astype(f16)),
            wb16=np.ascontiguousarray(wb.reshape(2, 128, W).astype(f16)),
            gW2=np.ascontiguousarray(gW2[:, perm]),
            gb2=np.ascontiguousarray(gb2[perm].reshape(1, N)),
        ))

    in_maps = []
    for c in range(8):
        p, hb = c // 2, c % 2
        m = dict(common)
        m.update(halves[hb])
        m.update(pairs[p])
        in_maps.append(m)
    return in_maps


def _make_runner(nc, n_cores=8):
    """Compile once; reusable executor for per-core input maps."""
    import jax
    from jax.sharding import Mesh, PartitionSpec
    from jax.experimental.shard_map import shard_map
    from concourse.bass2jax import (_bass_exec_p, install_neuronx_cc_hook,
                                    partition_id_tensor)

    install_neuronx_cc_hook()
    pname = nc.partition_id_tensor.name if nc.partition_id_tensor else None
    in_names, out_names, out_avals, zero_outs = [], [], [], []
    for alloc in nc.m.functions[0].allocations:
        if not isinstance(alloc, mybir.MemoryLocationSet):
            continue
        name = alloc.memorylocations[0].name
        if alloc.kind == "ExternalInput":
            if name != pname:
                in_names.append(name)
        elif alloc.kind == "ExternalOutput":
            out_names.append(name)
            shape = tuple(alloc.tensor_shape)
            dtype = mybir.dt.np(alloc.dtype)
            out_avals.append(jax.core.ShapedArray(shape, dtype))
            zero_outs.append(np.zeros(shape, dtype))
    n_params = len(in_names)
    n_outs = len(out_avals)
    all_names = in_names + out_names + ([pname] if pname else [])

    def _body(*args):
        operands = list(args)
        if pname is not None:
            operands.append(partition_id_tensor())
        outs = _bass_exec_p.bind(
            *operands, out_avals=tuple(out_avals), in_names=tuple(all_names),
            out_names=tuple(out_names), lowering_input_output_aliases=(),
            sim_require_finite=True, sim_require_nnan=True, nc=nc)
        return tuple(outs)

    devices = jax.devices()[:n_cores]
    mesh = Mesh(np.asarray(devices), ("core",))
    in_specs = (PartitionSpec("core"),) * (n_params + n_outs)
    out_specs = (PartitionSpec("core"),) * n_outs
    donate = tuple(range(n_params, n_params + n_outs))
    sharded = jax.jit(
        shard_map(_body, mesh=mesh, in_specs=in_specs, out_specs=out_specs,
                  check_rep=False),
        donate_argnums=donate, keep_unused=True)

    staged = {}

    def _concat(in_maps):
        return [
            np.concatenate([np.asarray(in_maps[c][in_names[i]])
                            for c in range(n_cores)], axis=0)
            for i in range(n_params)
        ]

    def run(in_maps):
        if in_maps is None:
            concat_in = staged["dev"]
        else:
            concat_in = _concat(in_maps)
        zeros = [np.zeros((n_cores * z.shape[0], *z.shape[1:]), z.dtype)
                 for z in zero_outs]
        outs = sharded(*concat_in, *zeros)
        arr = np.asarray(outs[0]).reshape(n_cores, *out_avals[0].shape)
        return [{out_names[0]: arr[c]} for c in range(n_cores)]

    def preload(in_maps):
        import jax
        staged["dev"] = [jax.device_put(a) for a in _concat(in_maps)]
        for a in staged["dev"]:
            a.block_until_ready()

    run.preload = preload
    return run


def kernel(**inputs):
    if "run" not in _cache:
        nc = _build()
        _cache["nc"] = nc
        _cache["run"] = _make_runner(nc)
    in_maps = _host_prep(**inputs)
    results = _cache["run"](in_maps)
    out = np.zeros((B, OUT), np.float32)
    for c in range(8):
        hb = c % 2
        out[hb * BH:(hb + 1) * BH] += results[c]["out"]
    return out
